# revision 8
# baseline (speedup 1.0000x reference)
"""ComputeMetrics (CE loss + entropy + top-k accuracy) Trainium2 kernel.

Splits the 4096-token × 32000-vocab logits across 8 NeuronCores along the
token axis (512 tokens/core).  Each core streams its 62.5 MB logit shard
once from HBM and computes, per token:
  s    = sum_v exp(l_v)                (ACT Exp with fused accumulate)
  dot  = sum_v exp(l_v) * l_v          (DVE scalar_tensor_tensor + accumulate)
  rank = #{v : l_v > l_label}          (DVE is_gt+accum / ACT Sign+accum,
                                        split between engines for balance)
  lse  = ln(s);  ce = (lse - l_label)*valid;  ent = (lse - dot/s)*valid
The label-logit gather, label shift and final scalar reductions are done
host-side (they are O(N) on 4096 tokens).
"""

import os
import sys

import numpy as np

for _p in ("/opt/trn_rl_repo", "/root/.axon_site/_ro/trn_rl_repo"):
    if os.path.isdir(_p) and _p not in sys.path:
        sys.path.append(_p)

import concourse.bass as bass
import concourse.mybir as mybir
import concourse.tile as tile
from concourse.bass_utils import run_bass_kernel_spmd

# Problem shape (hardcoded per contract)
B, S, V = 2, 2048, 32000
N = B * S  # 4096 flattened tokens
N_CORES = 8
TOK = N // N_CORES  # 512 tokens per core
P = 128  # SBUF partitions
NBLK = TOK // P  # 4 token blocks per core
FD = 8000  # vocab chunk (free dim) per instruction
NCH = V // FD  # 4 vocab chunks
K_LIST = (1, 5, 20)
IGNORE = -100

# Which engine counts rank for (block, chunk): True -> DVE tensor_scalar(is_gt),
# False -> ACT Sign.  6 of 16 chunks on DVE balances ACT and DVE busy time
# (ACT: 16 exp + 10 sign; DVE: 16 dot + 6 count -> both ~196us).
N_DVE_CNT = 6
_dve_set = {round(i * (NBLK * NCH) / N_DVE_CNT) for i in range(N_DVE_CNT)}
CNT_ON_DVE = [
    [(b * NCH + c) in _dve_set for c in range(NCH)] for b in range(NBLK)
]


def _split_multi_waits(nc):
    """This walrus build encodes at most one semaphore wait per instruction;
    hoist extra waits onto dedicated NoOps placed just before the owner."""
    fn = nc.m.functions[0]
    for blk in fn.blocks:
        changed = False
        new_list = []
        for inst in blk.instructions:
            si = inst.sync_info
            if si is not None and len(si.on_wait) > 1:
                waits = list(si.on_wait)
                for j, w in enumerate(waits[:-1]):
                    new_list.append(
                        mybir.InstNoOp(
                            name=f"{inst.name}-waitsplit-{j}",
                            engine=inst.engine,
                            ins=[],
                            outs=[],
                            sync_info=mybir.SyncInfo(on_wait=[w], on_update=[]),
                        )
                    )
                changed = True
                inst.sync_info = mybir.SyncInfo(
                    on_wait=[waits[-1]], on_update=list(si.on_update)
                )
            new_list.append(inst)
        if changed:
            blk.instructions = new_list


def _build_bass():
    dt = mybir.dt.float32
    A = mybir.AluOpType
    AF = mybir.ActivationFunctionType

    nc = bass.Bass()
    logits = nc.dram_tensor("logits", [TOK, V], dt, kind="ExternalInput")
    xlab = nc.dram_tensor("xlab", [P, NBLK], dt, kind="ExternalInput")
    nxlab = nc.dram_tensor("nxlab", [P, NBLK], dt, kind="ExternalInput")
    valid = nc.dram_tensor("valid", [P, NBLK], dt, kind="ExternalInput")
    ce_out = nc.dram_tensor("ce", [P, NBLK], dt, kind="ExternalOutput")
    ent_out = nc.dram_tensor("ent", [P, NBLK], dt, kind="ExternalOutput")
    cnt_out = nc.dram_tensor("cnt", [P, NBLK * NCH], dt, kind="ExternalOutput")
    sgn_out = nc.dram_tensor("sgn", [P, NBLK * NCH], dt, kind="ExternalOutput")

    with tile.TileContext(nc) as tc:
        with (
            tc.tile_pool(name="consts", bufs=1) as consts,
            tc.tile_pool(name="lpool", bufs=3) as lpool,
            tc.tile_pool(name="epool", bufs=2) as epool,
            tc.tile_pool(name="gpool", bufs=1) as gpool,
        ):
            # small input/const loads go on SWDGE (gpsimd) so the HWDGE
            # FIFO leads with the first big logits chunk
            xlab_t = consts.tile([P, NBLK], dt)
            nc.gpsimd.dma_start(out=xlab_t, in_=xlab[:, :])
            nxlab_t = consts.tile([P, NBLK], dt)
            nc.gpsimd.dma_start(out=nxlab_t, in_=nxlab[:, :])
            valid_t = consts.tile([P, NBLK], dt)
            nc.gpsimd.dma_start(out=valid_t, in_=valid[:, :])

            ce_t = consts.tile([P, NBLK], dt)
            ent_t = consts.tile([P, NBLK], dt)
            s_all = consts.tile([P, NBLK * NCH], dt)
            dot_all = consts.tile([P, NBLK * NCH], dt)
            cnt_t = consts.tile([P, NBLK * NCH], dt)
            sgn_t = consts.tile([P, NBLK * NCH], dt)
            nc.vector.memset(cnt_t, 0.0)
            nc.scalar.memzero(sgn_t)

            # pure streaming loop: no cross-engine reductions inside
            for b in range(NBLK):
                for c in range(NCH):
                    col = b * NCH + c
                    lt = lpool.tile([P, FD], dt, tag="lt")
                    nc.sync.dma_start(
                        out=lt,
                        in_=logits[b * P : (b + 1) * P, c * FD : (c + 1) * FD],
                    )
                    et = epool.tile([P, FD], dt, tag="et")
                    nc.scalar.activation(
                        out=et,
                        in_=lt,
                        func=AF.Exp,
                        accum_out=s_all[:, col : col + 1],
                    )
                    nc.vector.scalar_tensor_tensor(
                        out=et,
                        in0=et,
                        scalar=1.0,
                        in1=lt,
                        op0=A.mult,
                        op1=A.mult,
                        accum_out=dot_all[:, col : col + 1],
                    )
                    if CNT_ON_DVE[b][c]:
                        nc.vector.tensor_scalar(
                            out=lt,
                            in0=lt,
                            scalar1=xlab_t[:, b : b + 1],
                            scalar2=None,
                            op0=A.is_gt,
                            op1=A.add,
                            accum_out=cnt_t[:, col : col + 1],
                        )
                    else:
                        scr3 = gpool.tile([P, FD], mybir.dt.bfloat16, tag="sgnscr")
                        nc.scalar.activation(
                            out=scr3,
                            in_=lt,
                            func=AF.Sign,
                            bias=nxlab_t[:, b : b + 1],
                            scale=1.0,
                            accum_out=sgn_t[:, col : col + 1],
                        )

            # one-shot finalization on [P, NBLK] tiles
            s_sum = consts.tile([P, NBLK], dt)
            nc.vector.tensor_reduce(
                out=s_sum,
                in_=s_all.rearrange("p (b c) -> p b c", c=NCH),
                axis=mybir.AxisListType.X,
                op=A.add,
            )
            dot_sum = consts.tile([P, NBLK], dt)
            nc.vector.tensor_reduce(
                out=dot_sum,
                in_=dot_all.rearrange("p (b c) -> p b c", c=NCH),
                axis=mybir.AxisListType.X,
                op=A.add,
            )
            lse_all = consts.tile([P, NBLK], dt)
            nc.scalar.activation(out=lse_all, in_=s_sum, func=AF.Ln)
            inv_all = consts.tile([P, NBLK], dt)
            nc.vector.reciprocal(out=inv_all, in_=s_sum)
            mean_all = consts.tile([P, NBLK], dt)
            nc.vector.tensor_mul(mean_all, dot_sum, inv_all)
            nc.vector.tensor_sub(ce_t, lse_all, xlab_t)
            nc.vector.tensor_mul(ce_t, ce_t, valid_t)
            nc.vector.tensor_sub(ent_t, lse_all, mean_all)
            nc.vector.tensor_mul(ent_t, ent_t, valid_t)

            nc.sync.dma_start(out=ce_out[:, :], in_=ce_t)
            nc.sync.dma_start(out=ent_out[:, :], in_=ent_t)
            nc.sync.dma_start(out=cnt_out[:, :], in_=cnt_t)
            nc.sync.dma_start(out=sgn_out[:, :], in_=sgn_t)

    _split_multi_waits(nc)
    return nc


_NC_CACHE = None


def _get_nc():
    global _NC_CACHE
    if _NC_CACHE is None:
        _NC_CACHE = _build_bass()
    return _NC_CACHE


def kernel(logits, labels):
    logits = np.asarray(logits, dtype=np.float32)
    labels = np.asarray(labels)

    lg = logits.reshape(N, V)
    # shift labels left by one; pad with ignore_index
    shift = np.concatenate(
        [labels[:, 1:], np.full((B, 1), IGNORE, dtype=labels.dtype)], axis=1
    ).reshape(N)
    valid = shift >= 0
    safe = np.where(valid, shift, 0).astype(np.int64)
    xlab = lg[np.arange(N), safe].astype(np.float32)  # label logit per token
    valid_f = valid.astype(np.float32)

    in_maps = []
    for core in range(N_CORES):
        t0 = core * TOK
        # [P, NBLK] layout: element (p, b) is token t0 + b*P + p
        xl2 = np.ascontiguousarray(xlab[t0 : t0 + TOK].reshape(NBLK, P).T)
        vl2 = np.ascontiguousarray(valid_f[t0 : t0 + TOK].reshape(NBLK, P).T)
        in_maps.append(
            {
                "logits": np.ascontiguousarray(lg[t0 : t0 + TOK]),
                "xlab": xl2,
                "nxlab": np.ascontiguousarray(-xl2),
                "valid": vl2,
            }
        )

    nc = _get_nc()
    trace = os.environ.get("KERNEL_TRACE") == "1"
    res = run_bass_kernel_spmd(nc, in_maps, list(range(N_CORES)), trace=trace)
    if trace:
        kernel.last_exec_time_ns = res.exec_time_ns

    ce = np.empty(N, dtype=np.float32)
    ent = np.empty(N, dtype=np.float32)
    cnt = np.empty((N, NCH), dtype=np.float32)
    sgn = np.empty((N, NCH), dtype=np.float32)
    for core in range(N_CORES):
        t0 = core * TOK
        r = res.results[core]
        ce[t0 : t0 + TOK] = r["ce"].T.reshape(TOK)
        ent[t0 : t0 + TOK] = r["ent"].T.reshape(TOK)
        # [P, NBLK*NCH] -> (b, p, c)
        cnt[t0 : t0 + TOK] = (
            r["cnt"].reshape(P, NBLK, NCH).transpose(1, 0, 2).reshape(TOK, NCH)
        )
        sgn[t0 : t0 + TOK] = (
            r["sgn"].reshape(P, NBLK, NCH).transpose(1, 0, 2).reshape(TOK, NCH)
        )

    # rank = #{v : l_v > l_label}; DVE chunks counted directly, ACT chunks via
    # sign-sum: gt = (sum_sign + n_counted - n_equal) / 2 with n_equal = 1 in
    # the chunk holding the label itself.
    on_dve = np.array(CNT_ON_DVE, dtype=bool)  # [NBLK, NCH]
    blk_of_tok = (np.arange(N) % TOK) // P  # block index within core
    tok_on_dve = on_dve[blk_of_tok]  # [N, NCH]
    label_chunk = (safe // FD).astype(np.int64)  # chunk containing the label
    eq_act = ~tok_on_dve[np.arange(N), label_chunk]  # label's chunk on ACT?
    f_act = (~tok_on_dve).sum(axis=1) * FD
    cnt_dve = np.where(tok_on_dve, cnt.astype(np.float64), 0.0).sum(axis=1)
    sgn_act = np.where(~tok_on_dve, sgn.astype(np.float64), 0.0).sum(axis=1)
    rank = cnt_dve + (sgn_act + f_act - eq_act.astype(np.float64)) / 2.0
    rank = np.rint(rank)

    n_valid = np.float32(valid.sum())
    loss = np.float32(ce.sum(dtype=np.float64) / n_valid)
    accs = tuple(
        np.float32(((rank < k) & valid).sum(dtype=np.float64) / n_valid)
        for k in K_LIST
    )
    return (loss, ce, ent, *accs)


# revision 10
# speedup vs baseline: 1.0811x; 1.0811x over previous
"""ComputeMetrics (CE loss + entropy + top-k accuracy) Trainium2 kernel.

Splits the 4096-token × 32000-vocab logits across 8 NeuronCores along the
token axis (512 tokens/core).  Each core streams its 62.5 MB logit shard
once from HBM and computes, per token:
  s    = sum_v exp(l_v)                (ACT Exp with fused accumulate)
  dot  = sum_v exp(l_v) * l_v          (DVE scalar_tensor_tensor + accumulate)
  rank = #{v : l_v > l_label}          (DVE is_gt+accum / ACT Sign+accum,
                                        split between engines for balance)
  lse  = ln(s);  ce = (lse - l_label)*valid;  ent = (lse - dot/s)*valid
The label-logit gather, label shift and final scalar reductions are done
host-side (they are O(N) on 4096 tokens).
"""

import os
import sys

import numpy as np

for _p in ("/opt/trn_rl_repo", "/root/.axon_site/_ro/trn_rl_repo"):
    if os.path.isdir(_p) and _p not in sys.path:
        sys.path.append(_p)

import concourse.bass as bass
import concourse.mybir as mybir
import concourse.tile as tile
from concourse.bass_utils import run_bass_kernel_spmd

# Problem shape (hardcoded per contract)
B, S, V = 2, 2048, 32000
N = B * S  # 4096 flattened tokens
N_CORES = 8
TOK = N // N_CORES  # 512 tokens per core
P = 128  # SBUF partitions
NBLK = TOK // P  # 4 token blocks per core
FD = 8000  # vocab chunk (free dim) per instruction
NCH = V // FD  # 4 vocab chunks
K_LIST = (1, 5, 20)
IGNORE = -100

# Which engine counts rank for (block, chunk): True -> DVE tensor_scalar(is_gt),
# False -> ACT Sign.  6 of 16 chunks on DVE balances ACT and DVE busy time
# (ACT: 16 exp + 10 sign; DVE: 16 dot + 6 count -> both ~196us).
N_DVE_CNT = 6
_dve_set = {round(i * (NBLK * NCH) / N_DVE_CNT) for i in range(N_DVE_CNT)}
CNT_ON_DVE = [
    [(b * NCH + c) in _dve_set for c in range(NCH)] for b in range(NBLK)
]


def _split_multi_waits(nc):
    """This walrus build encodes at most one semaphore wait per instruction;
    hoist extra waits onto dedicated NoOps placed just before the owner."""
    fn = nc.m.functions[0]
    for blk in fn.blocks:
        changed = False
        new_list = []
        for inst in blk.instructions:
            si = inst.sync_info
            if si is not None and len(si.on_wait) > 1:
                waits = list(si.on_wait)
                for j, w in enumerate(waits[:-1]):
                    new_list.append(
                        mybir.InstNoOp(
                            name=f"{inst.name}-waitsplit-{j}",
                            engine=inst.engine,
                            ins=[],
                            outs=[],
                            sync_info=mybir.SyncInfo(on_wait=[w], on_update=[]),
                        )
                    )
                changed = True
                inst.sync_info = mybir.SyncInfo(
                    on_wait=[waits[-1]], on_update=list(si.on_update)
                )
            new_list.append(inst)
        if changed:
            blk.instructions = new_list


def _build_bass():
    dt = mybir.dt.float32
    A = mybir.AluOpType
    AF = mybir.ActivationFunctionType

    nc = bass.Bass()
    logits = nc.dram_tensor("logits", [TOK, V], dt, kind="ExternalInput")
    xlab = nc.dram_tensor("xlab", [P, NBLK], dt, kind="ExternalInput")
    nxlab = nc.dram_tensor("nxlab", [P, NBLK], dt, kind="ExternalInput")
    valid = nc.dram_tensor("valid", [P, NBLK], dt, kind="ExternalInput")
    ce_out = nc.dram_tensor("ce", [P, NBLK], dt, kind="ExternalOutput")
    ent_out = nc.dram_tensor("ent", [P, NBLK], dt, kind="ExternalOutput")
    cnt_out = nc.dram_tensor("cnt", [P, NBLK * NCH], dt, kind="ExternalOutput")
    sgn_out = nc.dram_tensor("sgn", [P, NBLK * NCH], dt, kind="ExternalOutput")

    with tile.TileContext(nc) as tc:
        with (
            tc.tile_pool(name="consts", bufs=1) as consts,
            tc.tile_pool(name="lpool", bufs=4) as lpool,
            tc.tile_pool(name="epool", bufs=2) as epool,
            tc.tile_pool(name="gpool", bufs=1) as gpool,
        ):
            # small input/const loads go on SWDGE (gpsimd) so the HWDGE
            # FIFO leads with the first big logits chunk
            xlab_t = consts.tile([P, NBLK], dt)
            nc.gpsimd.dma_start(out=xlab_t, in_=xlab[:, :])
            nxlab_t = consts.tile([P, NBLK], dt)
            nc.gpsimd.dma_start(out=nxlab_t, in_=nxlab[:, :])
            valid_t = consts.tile([P, NBLK], dt)
            nc.gpsimd.dma_start(out=valid_t, in_=valid[:, :])

            ce_t = consts.tile([P, NBLK], dt)
            ent_t = consts.tile([P, NBLK], dt)
            s_all = consts.tile([P, NBLK * NCH], dt)
            dot_all = consts.tile([P, NBLK * NCH], dt)
            cnt_t = consts.tile([P, NBLK * NCH], dt)
            sgn_t = consts.tile([P, NBLK * NCH], dt)
            nc.vector.memset(cnt_t, 0.0)
            nc.scalar.memzero(sgn_t)

            # pure streaming loop: no cross-engine reductions inside
            for b in range(NBLK):
                for c in range(NCH):
                    col = b * NCH + c
                    lt = lpool.tile([P, FD], dt, tag="lt")
                    nc.sync.dma_start(
                        out=lt,
                        in_=logits[b * P : (b + 1) * P, c * FD : (c + 1) * FD],
                    )
                    et = epool.tile([P, FD], dt, tag="et")
                    nc.scalar.activation(
                        out=et,
                        in_=lt,
                        func=AF.Exp,
                        accum_out=s_all[:, col : col + 1],
                    )
                    nc.vector.scalar_tensor_tensor(
                        out=et,
                        in0=et,
                        scalar=1.0,
                        in1=lt,
                        op0=A.mult,
                        op1=A.mult,
                        accum_out=dot_all[:, col : col + 1],
                    )
                    if CNT_ON_DVE[b][c]:
                        nc.vector.tensor_scalar(
                            out=lt,
                            in0=lt,
                            scalar1=xlab_t[:, b : b + 1],
                            scalar2=None,
                            op0=A.is_gt,
                            op1=A.add,
                            accum_out=cnt_t[:, col : col + 1],
                        )
                    else:
                        scr3 = gpool.tile([P, FD], mybir.dt.bfloat16, tag="sgnscr")
                        nc.scalar.activation(
                            out=scr3,
                            in_=lt,
                            func=AF.Sign,
                            bias=nxlab_t[:, b : b + 1],
                            scale=1.0,
                            accum_out=sgn_t[:, col : col + 1],
                        )

            # one-shot finalization on [P, NBLK] tiles
            s_sum = consts.tile([P, NBLK], dt)
            nc.vector.tensor_reduce(
                out=s_sum,
                in_=s_all.rearrange("p (b c) -> p b c", c=NCH),
                axis=mybir.AxisListType.X,
                op=A.add,
            )
            dot_sum = consts.tile([P, NBLK], dt)
            nc.vector.tensor_reduce(
                out=dot_sum,
                in_=dot_all.rearrange("p (b c) -> p b c", c=NCH),
                axis=mybir.AxisListType.X,
                op=A.add,
            )
            lse_all = consts.tile([P, NBLK], dt)
            nc.scalar.activation(out=lse_all, in_=s_sum, func=AF.Ln)
            inv_all = consts.tile([P, NBLK], dt)
            nc.vector.reciprocal(out=inv_all, in_=s_sum)
            mean_all = consts.tile([P, NBLK], dt)
            nc.vector.tensor_mul(mean_all, dot_sum, inv_all)
            nc.vector.tensor_sub(ce_t, lse_all, xlab_t)
            nc.vector.tensor_mul(ce_t, ce_t, valid_t)
            nc.vector.tensor_sub(ent_t, lse_all, mean_all)
            nc.vector.tensor_mul(ent_t, ent_t, valid_t)

            nc.sync.dma_start(out=ce_out[:, :], in_=ce_t)
            nc.sync.dma_start(out=ent_out[:, :], in_=ent_t)
            nc.sync.dma_start(out=cnt_out[:, :], in_=cnt_t)
            nc.sync.dma_start(out=sgn_out[:, :], in_=sgn_t)

    _split_multi_waits(nc)
    return nc


_NC_CACHE = None


def _get_nc():
    global _NC_CACHE
    if _NC_CACHE is None:
        _NC_CACHE = _build_bass()
    return _NC_CACHE


def kernel(logits, labels):
    logits = np.asarray(logits, dtype=np.float32)
    labels = np.asarray(labels)

    lg = logits.reshape(N, V)
    # shift labels left by one; pad with ignore_index
    shift = np.concatenate(
        [labels[:, 1:], np.full((B, 1), IGNORE, dtype=labels.dtype)], axis=1
    ).reshape(N)
    valid = shift >= 0
    safe = np.where(valid, shift, 0).astype(np.int64)
    xlab = lg[np.arange(N), safe].astype(np.float32)  # label logit per token
    valid_f = valid.astype(np.float32)

    in_maps = []
    for core in range(N_CORES):
        t0 = core * TOK
        # [P, NBLK] layout: element (p, b) is token t0 + b*P + p
        xl2 = np.ascontiguousarray(xlab[t0 : t0 + TOK].reshape(NBLK, P).T)
        vl2 = np.ascontiguousarray(valid_f[t0 : t0 + TOK].reshape(NBLK, P).T)
        in_maps.append(
            {
                "logits": np.ascontiguousarray(lg[t0 : t0 + TOK]),
                "xlab": xl2,
                "nxlab": np.ascontiguousarray(-xl2),
                "valid": vl2,
            }
        )

    nc = _get_nc()
    trace = os.environ.get("KERNEL_TRACE") == "1"
    res = run_bass_kernel_spmd(nc, in_maps, list(range(N_CORES)), trace=trace)
    if trace:
        kernel.last_exec_time_ns = res.exec_time_ns

    ce = np.empty(N, dtype=np.float32)
    ent = np.empty(N, dtype=np.float32)
    cnt = np.empty((N, NCH), dtype=np.float32)
    sgn = np.empty((N, NCH), dtype=np.float32)
    for core in range(N_CORES):
        t0 = core * TOK
        r = res.results[core]
        ce[t0 : t0 + TOK] = r["ce"].T.reshape(TOK)
        ent[t0 : t0 + TOK] = r["ent"].T.reshape(TOK)
        # [P, NBLK*NCH] -> (b, p, c)
        cnt[t0 : t0 + TOK] = (
            r["cnt"].reshape(P, NBLK, NCH).transpose(1, 0, 2).reshape(TOK, NCH)
        )
        sgn[t0 : t0 + TOK] = (
            r["sgn"].reshape(P, NBLK, NCH).transpose(1, 0, 2).reshape(TOK, NCH)
        )

    # rank = #{v : l_v > l_label}; DVE chunks counted directly, ACT chunks via
    # sign-sum: gt = (sum_sign + n_counted - n_equal) / 2 with n_equal = 1 in
    # the chunk holding the label itself.
    on_dve = np.array(CNT_ON_DVE, dtype=bool)  # [NBLK, NCH]
    blk_of_tok = (np.arange(N) % TOK) // P  # block index within core
    tok_on_dve = on_dve[blk_of_tok]  # [N, NCH]
    label_chunk = (safe // FD).astype(np.int64)  # chunk containing the label
    eq_act = ~tok_on_dve[np.arange(N), label_chunk]  # label's chunk on ACT?
    f_act = (~tok_on_dve).sum(axis=1) * FD
    cnt_dve = np.where(tok_on_dve, cnt.astype(np.float64), 0.0).sum(axis=1)
    sgn_act = np.where(~tok_on_dve, sgn.astype(np.float64), 0.0).sum(axis=1)
    rank = cnt_dve + (sgn_act + f_act - eq_act.astype(np.float64)) / 2.0
    rank = np.rint(rank)

    n_valid = np.float32(valid.sum())
    loss = np.float32(ce.sum(dtype=np.float64) / n_valid)
    accs = tuple(
        np.float32(((rank < k) & valid).sum(dtype=np.float64) / n_valid)
        for k in K_LIST
    )
    return (loss, ce, ent, *accs)
